# revision 1
# baseline (speedup 1.0000x reference)
"""Trainium2 Bass kernel for nn_ConvAttentionHybrid.

Math: the reference broadcasts the conv-sigmoid output f[s] along the embed
dim E, so q/k/v are affine (rank-1) in f.  The softmax logits collapse to
    l[s,t] = g[s]*f[t] + (terms constant in t),   g[s] = 0.5*(A*f[s] + C)
with A = rowsum(Wq).rowsum(Wk), C = bq.rowsum(Wk).  With h = f - 1/2:
    m(s) = Num(s)/Den(s)
    Den(s) = sum_n g^n/n! * W_n,          W_n = sum_t h_t^n
    Num(s) = sum_n g^n/n! * (W_{n+1} + W_n/2)
(the common e^{g/2} factor cancels in the ratio), and
    result = sv_sum*sum_s m(s)/(4*S) + bv_sum/4.
|g| <= ~1.1 and |h| <= 1/2 here, so 14 Taylor terms are exact to ~1e-12,
far below fp32 noise.  Each core computes f and the moments fully (cheap)
and evaluates m(s) for a 2048-row chunk of s selected by a per-core one-hot
matmul; the host sums the 8 partial outputs.
"""

import math
from contextlib import ExitStack

import numpy as np

import concourse.bass as bass
import concourse.tile as tile
from concourse import bacc, mybir
from concourse.bass_utils import run_bass_kernel_spmd

AF = mybir.ActivationFunctionType
OP = mybir.AluOpType
AX = mybir.AxisListType
F32 = mybir.dt.float32

NCORES = 8
NCOEF = 11            # Taylor coefficients n = 0..NCOEF-1
NMOM = NCOEF + 1      # moments W_0 .. W_NCOEF
JS = 16               # s-chunk columns per core (128*16 = 2048 s per core)
S_TOTAL = 16384

# feature flags (exotic instructions, enabled one by one after HW validation)
USE_TTR = False       # fused tensor_tensor_reduce for moments
USE_SCAN = False      # tensor_tensor_scan Horner
USE_GP_BUILDS = False # build scan operands on gpsimd


def _emit(ctx: ExitStack, tc: "tile.TileContext", d):
    nc = tc.nc
    pool = ctx.enter_context(tc.tile_pool(name="main", bufs=1))
    psum = ctx.enter_context(tc.tile_pool(name="ps", bufs=1, space="PSUM"))

    def T(name, shape):
        return pool.tile(shape, F32, tag=name, name=name)

    # ---------------- DMAs (sync: conv params first, then data; gpsimd: rest)
    wcols = T("wcols", [128, 5])                       # w00 w01 w10 w11 cb
    cw_ap = d["conv_w"].ap()
    nc.sync.dma_start(out=wcols[:, 0:4],
                      in_=bass.AP(cw_ap.tensor, cw_ap.offset, [[0, 128], [1, 4]]))
    cb_ap = d["conv_b"].ap()
    nc.sync.dma_start(out=wcols[:, 4:5],
                      in_=bass.AP(cb_ap.tensor, cb_ap.offset, [[0, 128], [1, 1]]))
    dataA = T("dataA", [128, 129]); dataB = T("dataB", [128, 129])
    nc.sync.dma_start(out=dataA[:, :], in_=d["data"].ap()[0:128, :])
    nc.sync.dma_start(out=dataB[:, :], in_=d["data"].ap()[1:129, :])
    e_sb = T("e_sb", [128, JS])
    nc.sync.dma_start(out=e_sb[:, :], in_=d["E"].ap())

    wq_sb = T("wq_sb", [4, 4]); wk_sb = T("wk_sb", [4, 4]); wv_sb = T("wv_sb", [4, 4])
    bq_sb = T("bq_sb", [4, 1]); bv_row = T("bv_row", [1, 4])
    nc.gpsimd.dma_start(out=wq_sb[:, :], in_=d["Wq"].ap())
    nc.gpsimd.dma_start(out=wk_sb[:, :], in_=d["Wk"].ap())
    nc.gpsimd.dma_start(out=wv_sb[:, :], in_=d["Wv"].ap())
    nc.gpsimd.dma_start(out=bq_sb[:, :], in_=d["bq"].ap().rearrange("a -> a ()"))
    nc.gpsimd.dma_start(out=bv_row[:, :], in_=d["bv"].ap().rearrange("a -> () a"))
    invf_sb = T("invf_sb", [1, 16])
    nc.gpsimd.dma_start(out=invf_sb[:, :], in_=d["invf"].ap())

    # ---------------- early constants / table prefetch ---------------------
    z0 = T("z0", [128, 128]); ones4 = T("ones4", [4, 1]); onescol = T("onescol", [128, 1])
    ones1row = T("ones1row", [1, 128])
    nc.vector.memset(z0[:, :], 0.0)
    nc.vector.memset(ones4[:, :], 1.0)
    nc.vector.memset(onescol[:, :], 1.0)
    nc.vector.memset(ones1row[:, :], 1.0)
    dums = T("dums", [4, 1])
    nc.scalar.activation(dums[:, :], ones4[:, :], AF.Sigmoid, bias=0.0, scale=1.0)

    # ---------------- conv + sigmoid -> f [128,128] ------------------------
    c1 = T("c1", [128, 128]); c2 = T("c2", [128, 128])
    c3 = T("c3", [128, 128]); pre = T("pre", [128, 128])
    f = T("f", [128, 128])
    with tc.high_priority():
        nc.vector.scalar_tensor_tensor(c1[:, :], dataA[:, 0:128], wcols[:, 0:1], z0[:, :], OP.mult, OP.add)
        nc.vector.scalar_tensor_tensor(c2[:, :], dataA[:, 1:129], wcols[:, 1:2], c1[:, :], OP.mult, OP.add)
        nc.vector.scalar_tensor_tensor(c3[:, :], dataB[:, 0:128], wcols[:, 2:3], c2[:, :], OP.mult, OP.add)
        nc.vector.scalar_tensor_tensor(pre[:, :], dataB[:, 1:129], wcols[:, 3:4], c3[:, :], OP.mult, OP.add)
        nc.scalar.activation(f[:, :], pre[:, :], AF.Sigmoid, bias=wcols[:, 4:5], scale=1.0)

    # ---------------- A/C/sv/bv scalars (vector fills the sigmoid bubble) ---
    qk_ps = psum.tile([4, 4], F32, tag="qk", name="qk")
    nc.tensor.matmul(qk_ps[:, :], wq_sb[:, :], wk_sb[:, :], start=True, stop=True)
    bqk_ps = psum.tile([1, 4], F32, tag="bqk", name="bqk")
    nc.tensor.matmul(bqk_ps[:, :], bq_sb[:, :], wk_sb[:, :], start=True, stop=True)
    small = T("small", [4, 2])
    nc.vector.reduce_sum(small[0:4, 0:1], qk_ps[:, :], axis=AX.X)
    nc.vector.reduce_sum(small[0:4, 1:2], wv_sb[:, :], axis=AX.X)
    c_sb = T("c_sb", [1, 1])
    nc.vector.reduce_sum(c_sb[:, :], bqk_ps[:, :], axis=AX.X)
    bvs_sb = T("bvs_sb", [1, 1])
    nc.vector.reduce_sum(bvs_sb[:, :], bv_row[:, :], axis=AX.X)
    srow_ps = psum.tile([1, 2], F32, tag="srow", name="srow")   # [A, sv_sum]
    nc.tensor.matmul(srow_ps[:, :], ones4[:, :], small[0:4, 0:2], start=True, stop=True)
    svs_sb = T("svs_sb", [1, 1])
    nc.vector.tensor_copy(svs_sb[:, :], srow_ps[0:1, 1:2])
    prow = T("prow", [1, 2])                           # [halfA, halfC]
    nc.vector.tensor_scalar_mul(prow[0:1, 0:1], srow_ps[0:1, 0:1], 0.5)
    nc.vector.tensor_scalar_mul(prow[0:1, 1:2], c_sb[:, :], 0.5)
    pbc_ps = psum.tile([128, 2], F32, tag="pbcp", name="pbcp")
    nc.tensor.matmul(pbc_ps[:, :], ones1row[:, :], prow[0:1, :], start=True, stop=True)
    pbc = T("pbc", [128, 2])
    nc.vector.tensor_copy(pbc[:, :], pbc_ps[:, :])

    # ---------------- per-core chunk: g = halfA*f_s + halfC ----------------
    chunk_ps = psum.tile([128, JS], F32, tag="chunk", name="chunk")
    nc.tensor.matmul(chunk_ps[:, :], f[:, :], e_sb[:, :], start=True, stop=True)
    g = T("g", [128, JS])
    nc.scalar.activation(g[:, :], chunk_ps[:, :], AF.Identity, bias=pbc[:, 1:2], scale=pbc[:, 0:1])

    # ---------------- moments W_n = sum h^n  (h = f - 1/2) -----------------
    # vector: power chain only.  PE: per-power partition sums into rows of P
    # (row j holds the column sums of W_{NMOM-1-j}).  One vector reduce +
    # a tiny DMA transpose turn P into the wrow coefficient row.
    h = T("h", [128, 128])
    nc.vector.tensor_scalar(h[:, :], f[:, :], 0.5, None, OP.subtract)
    wacc = T("wacc", [128, 16])
    acc_dst = T("acc_dst", [128, 128])
    nc.vector.memset(wacc[:, NMOM - 1:NMOM], 128.0)    # W_0 partial
    nc.vector.reduce_sum(wacc[:, NMOM - 2:NMOM - 1], h[:, :], axis=AX.X)
    pw = {1: h}
    for n in range(2, NMOM):
        pw[n] = T(f"pw{n}", [128, 128])
        a, b = (n - 2, 2) if n > 3 else (1, n - 1)     # pw2=h*h, pw3=h2*h, pw_n=pw_{n-2}*pw2
        nc.vector.tensor_mul(pw[n][:, :], pw[a][:, :], pw[b][:, :])
        col = wacc[:, NMOM - 1 - n:NMOM - n]
        if n % 2 == 0:
            nc.scalar.activation(acc_dst[:, :], pw[n][:, :], AF.Copy, bias=0.0,
                                 scale=1.0, accum_out=col)
        else:
            nc.vector.reduce_sum(col, pw[n][:, :], axis=AX.X)
    wrow_ps = psum.tile([1, NMOM], F32, tag="wrowp", name="wrowp")
    nc.tensor.matmul(wrow_ps[:, :], onescol[:, :], wacc[:, 0:NMOM], start=True, stop=True)
    wrow = T("wrow_sb", [1, NMOM])                     # col j = W_{NMOM-1-j}
    nc.vector.tensor_copy(wrow[:, :], wrow_ps[:, :])

    # ---------------- Taylor coefficients (reversed, Horner order) ---------
    coeff = T("coeff", [1, 2 * NCOEF])
    tmp14 = T("tmp14", [1, NCOEF])
    nc.vector.tensor_mul(coeff[0:1, 0:NCOEF], wrow[0:1, 1:NMOM], invf_sb[0:1, 0:NCOEF])
    nc.vector.scalar_tensor_tensor(tmp14[:, :], wrow[0:1, 1:NMOM], 0.5, wrow[0:1, 0:NCOEF], OP.mult, OP.add)
    nc.vector.tensor_mul(coeff[0:1, NCOEF:2 * NCOEF], tmp14[:, :], invf_sb[0:1, 0:NCOEF])
    coeffb_ps = psum.tile([128, 2 * NCOEF], F32, tag="coefbp", name="coefbp")
    nc.tensor.matmul(coeffb_ps[:, :], ones1row[:, :], coeff[0:1, :], start=True, stop=True)
    coeffb = T("coeffb", [128, 2 * NCOEF])
    nc.vector.tensor_copy(coeffb[:, :], coeffb_ps[:, :])

    # ---------------- fused Den/Num Horner on [128, 32] --------------------
    # t-form Horner: with t = s*g the step becomes t = (t + c)*g — one fused
    # STT per chain per coefficient; the trailing *g cancels in Num/Den.
    td = T("td", [128, JS]); tn = T("tn", [128, JS])
    nc.vector.scalar_tensor_tensor(td[:, :], z0[:, 0:JS], coeffb[:, 0:1], g[:, :], OP.add, OP.mult)
    nc.vector.scalar_tensor_tensor(tn[:, :], z0[:, 0:JS], coeffb[:, NCOEF:NCOEF + 1], g[:, :], OP.add, OP.mult)
    for k in range(1, NCOEF):
        nc.vector.scalar_tensor_tensor(td[:, :], td[:, :], coeffb[:, k:k + 1], g[:, :], OP.add, OP.mult)
        nc.vector.scalar_tensor_tensor(tn[:, :], tn[:, :], coeffb[:, NCOEF + k:NCOEF + k + 1], g[:, :], OP.add, OP.mult)
    den = td[:, :]
    num = tn[:, :]

    # ---------------- m = Num/Den, partial sum -----------------------------
    rden = T("rden", [128, JS])
    nc.vector.reciprocal(rden[:, :], den)
    mprod = T("mprod", [128, JS])
    mcol = T("mcol", [128, 1])
    nc.vector.tensor_mul(mprod[:, :], num, rden[:, :])
    nc.vector.reduce_sum(mcol[:, :], mprod[:, :], axis=AX.X)
    msum_ps = psum.tile([1, 1], F32, tag="msum", name="msum")
    nc.tensor.matmul(msum_ps[:, :], onescol[:, :], mcol[:, :], start=True, stop=True)

    # out = sv_sum * msum / (4*S) + bv_sum / (4*ncores)
    msum_sb = T("msum_sb", [1, 1])
    nc.vector.tensor_copy(msum_sb[:, :], msum_ps[:, :])
    ta = T("ta", [1, 1])
    nc.vector.tensor_mul(ta[:, :], msum_sb[:, :], svs_sb[:, :])
    bvt = T("bvt", [1, 1])
    nc.vector.tensor_scalar_mul(bvt[:, :], bvs_sb[:, :], 1.0 / (4.0 * NCORES))
    out_sb = T("out_sb", [1, 1])
    nc.vector.scalar_tensor_tensor(out_sb[:, :], ta[:, :], 1.0 / (4.0 * S_TOTAL), bvt[:, :], OP.mult, OP.add)
    nc.sync.dma_start(out=d["out"].ap(), in_=out_sb[:, :])


def build_nc():
    nc = bacc.Bacc("TRN2", target_bir_lowering=False, debug=False,
                   enable_asserts=False, num_devices=NCORES)
    d = {}
    d["data"] = nc.dram_tensor("data", [129, 129], F32, kind="ExternalInput")
    d["conv_w"] = nc.dram_tensor("conv_w", [1, 1, 2, 2], F32, kind="ExternalInput")
    d["conv_b"] = nc.dram_tensor("conv_b", [1], F32, kind="ExternalInput")
    d["Wq"] = nc.dram_tensor("Wq", [4, 4], F32, kind="ExternalInput")
    d["bq"] = nc.dram_tensor("bq", [4], F32, kind="ExternalInput")
    d["Wk"] = nc.dram_tensor("Wk", [4, 4], F32, kind="ExternalInput")
    d["Wv"] = nc.dram_tensor("Wv", [4, 4], F32, kind="ExternalInput")
    d["bv"] = nc.dram_tensor("bv", [4], F32, kind="ExternalInput")
    d["E"] = nc.dram_tensor("E", [128, JS], F32, kind="ExternalInput")
    d["invf"] = nc.dram_tensor("invf", [1, 16], F32, kind="ExternalInput")
    d["out"] = nc.dram_tensor("out", [1, 1], F32, kind="ExternalOutput")
    with tile.TileContext(nc) as tc:
        with ExitStack() as ctx:
            _emit(ctx, tc, d)
    nc.compile()
    return nc


_NC = None


def _get_nc():
    global _NC
    if _NC is None:
        _NC = build_nc()
    return _NC


def make_in_maps(inputs):
    invf = np.zeros((1, 16), np.float32)
    for k in range(NCOEF):
        invf[0, k] = 1.0 / math.factorial(NCOEF - 1 - k)
    base = {
        "data": np.ascontiguousarray(inputs["data"], np.float32),
        "conv_w": np.ascontiguousarray(inputs["conv_w"], np.float32),
        "conv_b": np.ascontiguousarray(inputs["conv_b"], np.float32),
        "Wq": np.ascontiguousarray(inputs["Wq"], np.float32),
        "bq": np.ascontiguousarray(inputs["bq"], np.float32),
        "Wk": np.ascontiguousarray(inputs["Wk"], np.float32),
        "Wv": np.ascontiguousarray(inputs["Wv"], np.float32),
        "bv": np.ascontiguousarray(inputs["bv"], np.float32),
        "invf": invf,
    }
    in_maps = []
    for c in range(NCORES):
        e = np.zeros((128, JS), np.float32)
        e[16 * c + np.arange(JS), np.arange(JS)] = 1.0
        in_maps.append(dict(base, E=e))
    return in_maps


def run_on_hw(inputs, trace=False, **kw):
    nc = _get_nc()
    res = run_bass_kernel_spmd(nc, make_in_maps(inputs),
                               core_ids=list(range(NCORES)), trace=trace, **kw)
    total = np.float64(0.0)
    for r in res.results:
        total += np.float64(r["out"][0, 0])
    return np.float32(total), res


def kernel(**inputs) -> np.ndarray:
    out, _ = run_on_hw(inputs, trace=False)
    return out



# revision 10
# speedup vs baseline: 1.0505x; 1.0505x over previous
"""Trainium2 Bass kernel for nn_ConvAttentionHybrid.

Math: the reference broadcasts the conv-sigmoid output f[s] along the embed
dim E, so q/k/v are affine (rank-1) in f.  The softmax weights collapse to
    w[s,t] ~ exp(g[s]*h[t]),   g = A/4*u + (A/4 + C/2),  u = 2h = 2f-1
with u = tanh(z/2) for conv pre-activation z (sigmoid(z)-1/2 = tanh(z/2)/2),
A = rowsum(Wq).rowsum(Wk), C = bq.rowsum(Wk).  With moments W'_n = sum_t u^n:
    Den(s) = sum_n g^n/n! 2^-n W'_n
    Num(s) = sum_n g^n/n! 2^-(n+1) (W'_{n+1} + W'_n)      (= Num_f directly)
    m(s)   = Num/Den,   result = sv_sum*mean_s m(s)/4 + bv_sum/4.
|g*h| <= ~0.5 here so 5 Taylor terms (n=0..4) are exact to ~5e-7, far below
the 2e-2 gate.  Each core computes u and the moments fully (cheap) and
evaluates m(s) for a 2048-row chunk of s selected by a per-core one-hot
matmul; the host applies the final affine and sums the partial outputs.
"""

import os

import numpy as np

from contextlib import ExitStack

K_SCALAR_DMA = os.environ.get("K_SCALAR_DMA", "1") == "1"
K_BYPASS = os.environ.get("K_BYPASS", "1") == "1"
K_ACTACCUM = os.environ.get("K_ACTACCUM", "1") == "1"
# tensor_tensor_reduce hard-crashes the exec unit on HW (NRT_EXEC_UNIT_
# UNRECOVERABLE) — keep off.
K_TTR = os.environ.get("K_TTR", "0") == "1"

import concourse.bass as bass
import concourse.tile as tile
from concourse import bacc, mybir
from concourse.bass_utils import run_bass_kernel_spmd

AF = mybir.ActivationFunctionType
OP = mybir.AluOpType
AX = mybir.AxisListType
F32 = mybir.dt.float32

NCORES = 8
NCOEF = 5             # Taylor terms n = 0..NCOEF-1
NMOM = NCOEF + 1      # moments W'_0 .. W'_NCOEF
JS = 16               # s-chunk columns per core (128*16 = 2048 s per core)
S_TOTAL = 16384

# params tensor column layout ([128, PCOLS] fp32, single DMA)
PC_E = 0              # cols 0:16  one-hot chunk selector (per-core)
PC_W00 = 16           # conv taps, broadcast down partitions
PC_W01 = 17
PC_W10 = 18
PC_W11 = 19
PC_CBH = 20           # 0.5*conv_b broadcast (tanh bias)
PC_GA = 21            # g scale broadcast
PC_GC = 22            # g bias broadcast
PC_CA = 25            # row 0, cols 25:30 : den coeff scales invf_k*2^-k
PC_CB = 30            # row 0, cols 30:35 : num coeff scales invf_k*2^-(k+1)
PCOLS = 35


def _emit(ctx: ExitStack, tc: "tile.TileContext", d):
    nc = tc.nc
    pool = ctx.enter_context(tc.tile_pool(name="main", bufs=1))
    psum = ctx.enter_context(tc.tile_pool(name="ps", bufs=1, space="PSUM"))

    def T(name, shape):
        return pool.tile(shape, F32, tag=name, name=name)

    # ---------------- DMAs: one per queue, data first -----------------------
    dataA = T("dataA", [128, 129])
    dataB = T("dataB", [128, 129])
    prm = T("prm", [128, PCOLS])
    nc.sync.dma_start(out=dataA[:, :], in_=d["data"].ap()[0:128, :])
    nc.gpsimd.dma_start(out=dataB[:, :], in_=d["data"].ap()[1:129, :])
    if K_SCALAR_DMA:
        nc.scalar.dma_start(out=prm[:, :], in_=d["params"].ap())
    else:
        nc.sync.dma_start(out=prm[:, :], in_=d["params"].ap())

    # ---------------- constants + act-table warmup --------------------------
    onescol = T("onescol", [128, 1])
    ones1row = T("ones1row", [1, 128])
    wacc = T("wacc", [128, NMOM + NCOEF])   # cols 0:6 W'_n, cols 6:11 W'_k+W'_{k+1}
    nc.vector.memset(onescol[:, :], 1.0)
    nc.vector.memset(ones1row[:, :], 1.0)
    nc.vector.memset(wacc[:, 0:1], 128.0)          # W'_0 partial (128*128=S)
    z0 = None
    if not K_BYPASS:
        z0 = T("z0", [128, 128])
        nc.vector.memset(z0[:, :], 0.0)
    dum = T("dum", [1, 1])
    nc.scalar.activation(dum[:, :], onescol[0:1, 0:1], AF.Tanh, bias=0.0, scale=1.0)

    # ---------------- conv -> pre (vector, two parallel 2-chains) -----------
    q1 = T("q1", [128, 128]); p1 = T("p1", [128, 128])
    q2 = T("q2", [128, 128]); p2 = T("p2", [128, 128])
    pre = T("pre", [128, 128])
    if K_BYPASS:
        nc.vector.scalar_tensor_tensor(q1[:, :], dataA[:, 0:128], prm[:, PC_W00:PC_W00 + 1],
                                       dataA[:, 0:128], OP.mult, OP.bypass)
        nc.vector.scalar_tensor_tensor(p1[:, :], dataB[:, 0:128], prm[:, PC_W10:PC_W10 + 1],
                                       dataB[:, 0:128], OP.mult, OP.bypass)
    else:
        nc.vector.scalar_tensor_tensor(q1[:, :], dataA[:, 0:128], prm[:, PC_W00:PC_W00 + 1],
                                       z0[:, :], OP.mult, OP.add)
        nc.vector.scalar_tensor_tensor(p1[:, :], dataB[:, 0:128], prm[:, PC_W10:PC_W10 + 1],
                                       z0[:, :], OP.mult, OP.add)
    nc.vector.scalar_tensor_tensor(q2[:, :], dataA[:, 1:129], prm[:, PC_W01:PC_W01 + 1],
                                   q1[:, :], OP.mult, OP.add)
    nc.vector.scalar_tensor_tensor(p2[:, :], dataB[:, 1:129], prm[:, PC_W11:PC_W11 + 1],
                                   p1[:, :], OP.mult, OP.add)
    nc.vector.tensor_add(pre[:, :], q2[:, :], p2[:, :])

    # ---------------- u = tanh(pre/2 + cb/2) = 2f-1, accum -> W'_1 ----------
    u = T("u", [128, 128])
    if K_ACTACCUM:
        nc.scalar.activation(u[:, :], pre[:, :], AF.Tanh,
                             bias=prm[:, PC_CBH:PC_CBH + 1], scale=0.5,
                             accum_out=wacc[:, 1:2])
    else:
        nc.scalar.activation(u[:, :], pre[:, :], AF.Tanh,
                             bias=prm[:, PC_CBH:PC_CBH + 1], scale=0.5)
        nc.vector.reduce_sum(wacc[:, 1:2], u[:, :], axis=AX.X)

    # ---------------- moments W'_2..W'_5 (fused mul+reduce on vector) -------
    u2 = T("u2", [128, 128]); u3 = T("u3", [128, 128])
    s4 = T("s4", [128, 128]); s5 = T("s5", [128, 128])
    if K_TTR:
        nc.vector.tensor_tensor_reduce(u2[:, :], u[:, :], u[:, :], 1.0, 0.0,
                                       OP.mult, OP.add, wacc[:, 2:3])
        nc.vector.tensor_tensor_reduce(u3[:, :], u2[:, :], u[:, :], 1.0, 0.0,
                                       OP.mult, OP.add, wacc[:, 3:4])
        nc.vector.tensor_tensor_reduce(s4[:, :], u2[:, :], u2[:, :], 1.0, 0.0,
                                       OP.mult, OP.add, wacc[:, 4:5])
        nc.vector.tensor_tensor_reduce(s5[:, :], u2[:, :], u3[:, :], 1.0, 0.0,
                                       OP.mult, OP.add, wacc[:, 5:6])
    else:
        nc.vector.tensor_mul(u2[:, :], u[:, :], u[:, :])
        nc.vector.tensor_mul(u3[:, :], u2[:, :], u[:, :])
        nc.vector.tensor_mul(s4[:, :], u2[:, :], u2[:, :])
        nc.vector.tensor_mul(s5[:, :], u2[:, :], u3[:, :])
        nc.vector.reduce_sum(wacc[:, 2:3], u2[:, :], axis=AX.X)
        nc.vector.reduce_sum(wacc[:, 3:4], u3[:, :], axis=AX.X)
        nc.vector.reduce_sum(wacc[:, 4:5], s4[:, :], axis=AX.X)
        nc.vector.reduce_sum(wacc[:, 5:6], s5[:, :], axis=AX.X)

    # ---------------- per-core chunk select + g (PE + scalar) ---------------
    chunk_ps = psum.tile([128, JS], F32, tag="chunk", name="chunk")
    nc.tensor.matmul(chunk_ps[:, :], u[:, :], prm[:, PC_E:PC_E + JS],
                     start=True, stop=True)
    g = T("g", [128, JS])
    nc.scalar.activation(g[:, :], chunk_ps[:, :], AF.Identity,
                         bias=prm[:, PC_GC:PC_GC + 1], scale=prm[:, PC_GA:PC_GA + 1])

    # ---------------- coefficient rows ---------------------------------------
    # wrow[0, n] = W'_n ; cd_k = W'_k*cA_k ; cn_k = (W'_k + W'_{k+1})*cB_k
    nc.vector.tensor_add(wacc[:, NMOM:NMOM + NCOEF], wacc[:, 0:NCOEF], wacc[:, 1:NMOM])
    wrow_ps = psum.tile([1, NMOM + NCOEF], F32, tag="wrowp", name="wrowp")
    nc.tensor.matmul(wrow_ps[:, :], onescol[:, :], wacc[:, :], start=True, stop=True)
    coeff = T("coeff", [1, 2 * NCOEF])
    nc.vector.tensor_mul(coeff[0:1, 0:NCOEF], wrow_ps[0:1, 0:NCOEF],
                         prm[0:1, PC_CA:PC_CA + NCOEF])
    nc.vector.tensor_mul(coeff[0:1, NCOEF:2 * NCOEF], wrow_ps[0:1, NMOM:NMOM + NCOEF],
                         prm[0:1, PC_CB:PC_CB + NCOEF])
    coeffb_ps = psum.tile([128, 2 * NCOEF], F32, tag="coefbp", name="coefbp")
    nc.tensor.matmul(coeffb_ps[:, :], ones1row[:, :], coeff[0:1, :],
                     start=True, stop=True)

    # ---------------- fused Den/Num Horner on [128, 16] ----------------------
    # t-form: t = (t + c)*g each step; the trailing *g cancels in Num/Den.
    td = T("td", [128, JS]); tn = T("tn", [128, JS])
    if K_BYPASS:
        nc.vector.scalar_tensor_tensor(td[:, :], g[:, :], coeffb_ps[:, NCOEF - 1:NCOEF],
                                       g[:, :], OP.mult, OP.bypass)
        nc.vector.scalar_tensor_tensor(tn[:, :], g[:, :], coeffb_ps[:, 2 * NCOEF - 1:2 * NCOEF],
                                       g[:, :], OP.mult, OP.bypass)
    else:
        nc.vector.scalar_tensor_tensor(td[:, :], z0[:, 0:JS], coeffb_ps[:, NCOEF - 1:NCOEF],
                                       g[:, :], OP.add, OP.mult)
        nc.vector.scalar_tensor_tensor(tn[:, :], z0[:, 0:JS], coeffb_ps[:, 2 * NCOEF - 1:2 * NCOEF],
                                       g[:, :], OP.add, OP.mult)
    for k in range(NCOEF - 2, -1, -1):
        nc.vector.scalar_tensor_tensor(td[:, :], td[:, :], coeffb_ps[:, k:k + 1],
                                       g[:, :], OP.add, OP.mult)
        nc.vector.scalar_tensor_tensor(tn[:, :], tn[:, :], coeffb_ps[:, NCOEF + k:NCOEF + k + 1],
                                       g[:, :], OP.add, OP.mult)

    # ---------------- m = Num/Den, per-partition partial sums ---------------
    rden = T("rden", [128, JS])
    nc.vector.reciprocal(rden[:, :], td[:, :])
    mprod = T("mprod", [128, JS])
    mcol = T("mcol", [128, 1])
    if K_TTR:
        nc.vector.tensor_tensor_reduce(mprod[:, :], tn[:, :], rden[:, :], 1.0, 0.0,
                                       OP.mult, OP.add, mcol[:, :])
    else:
        nc.vector.tensor_mul(mprod[:, :], tn[:, :], rden[:, :])
        nc.vector.reduce_sum(mcol[:, :], mprod[:, :], axis=AX.X)
    nc.sync.dma_start(out=d["out"].ap(), in_=mcol[:, :])


def build_nc():
    nc = bacc.Bacc("TRN2", target_bir_lowering=False, debug=False,
                   enable_asserts=False, num_devices=NCORES)
    d = {}
    d["data"] = nc.dram_tensor("data", [129, 129], F32, kind="ExternalInput")
    d["params"] = nc.dram_tensor("params", [128, PCOLS], F32, kind="ExternalInput")
    d["out"] = nc.dram_tensor("out", [128, 1], F32, kind="ExternalOutput")
    with tile.TileContext(nc) as tc:
        with ExitStack() as ctx:
            _emit(ctx, tc, d)
    nc.compile()
    return nc


_NC = None


def _get_nc():
    global _NC
    if _NC is None:
        _NC = build_nc()
    return _NC


def _factorial(n):
    r = 1
    for i in range(2, n + 1):
        r *= i
    return r


def _host_derived(inputs):
    Wq = np.asarray(inputs["Wq"], np.float64)
    Wk = np.asarray(inputs["Wk"], np.float64)
    Wv = np.asarray(inputs["Wv"], np.float64)
    bq = np.asarray(inputs["bq"], np.float64)
    bv = np.asarray(inputs["bv"], np.float64)
    rq = Wq.sum(1); rk = Wk.sum(1); rv = Wv.sum(1)
    A = float(rq @ rk)
    C = float(bq @ rk)
    ga = A / 4.0
    gc = A / 4.0 + C / 2.0
    alpha = rv.sum() / (4.0 * S_TOTAL)
    beta = bv.sum() / 4.0
    return ga, gc, alpha, beta


def make_in_maps(inputs):
    cw = np.asarray(inputs["conv_w"], np.float64)[0, 0]
    cbh = 0.5 * float(np.asarray(inputs["conv_b"], np.float64)[0])
    ga, gc, _, _ = _host_derived(inputs)

    base_p = np.zeros((128, PCOLS), np.float32)
    base_p[:, PC_W00] = cw[0, 0]
    base_p[:, PC_W01] = cw[0, 1]
    base_p[:, PC_W10] = cw[1, 0]
    base_p[:, PC_W11] = cw[1, 1]
    base_p[:, PC_CBH] = cbh
    base_p[:, PC_GA] = ga
    base_p[:, PC_GC] = gc
    for k in range(NCOEF):
        base_p[0, PC_CA + k] = 2.0 ** -k / _factorial(k)
        base_p[0, PC_CB + k] = 2.0 ** -(k + 1) / _factorial(k)

    data = np.ascontiguousarray(inputs["data"], np.float32)
    in_maps = []
    for c in range(NCORES):
        p = base_p.copy()
        p[JS * c + np.arange(JS), np.arange(JS)] = 1.0
        in_maps.append({"data": data, "params": p})
    return in_maps


def run_on_hw(inputs, trace=False, **kw):
    nc = _get_nc()
    res = run_bass_kernel_spmd(nc, make_in_maps(inputs),
                               core_ids=list(range(NCORES)), trace=trace, **kw)
    _, _, alpha, beta = _host_derived(inputs)
    total = np.float64(0.0)
    for r in res.results:
        total += np.asarray(r["out"], np.float64).sum()
    return np.float32(alpha * total + beta), res


def kernel(**inputs) -> np.ndarray:
    out, _ = run_on_hw(inputs, trace=False)
    return out


# revision 11
# speedup vs baseline: 1.4519x; 1.3821x over previous
"""Trainium2 Bass kernel for nn_ConvAttentionHybrid.

Math: the reference broadcasts the conv-sigmoid output f[s] along the embed
dim E, so q/k/v are affine (rank-1) in f.  The softmax weights collapse to
    w[s,t] ~ exp(g[s]*h[t]),   g = A/4*u + (A/4 + C/2),  u = 2h = 2f-1
with u = tanh(z/2) for conv pre-activation z (sigmoid(z)-1/2 = tanh(z/2)/2),
A = rowsum(Wq).rowsum(Wk), C = bq.rowsum(Wk).  With moments W'_n = sum_t u^n:
    Den(s) = sum_n g^n/n! 2^-n W'_n
    Num(s) = sum_n g^n/n! 2^-(n+1) (W'_{n+1} + W'_n)      (= Num_f directly)
    m(s)   = Num/Den,   result = sv_sum*mean_s m(s)/4 + bv_sum/4.
|g*h| <= ~0.5 here so 5 Taylor terms (n=0..4) are exact to ~5e-7, far below
the 2e-2 gate.  Each core computes u and the moments fully (cheap) and
evaluates m(s) for a 2048-row chunk of s selected by a per-core one-hot
matmul; the host applies the final affine and sums the partial outputs.
"""

import os

import numpy as np

from contextlib import ExitStack

import concourse.bass as bass
import concourse.tile as tile
from concourse import bacc, mybir
from concourse.bass_utils import run_bass_kernel_spmd

AF = mybir.ActivationFunctionType
OP = mybir.AluOpType
AX = mybir.AxisListType
F32 = mybir.dt.float32

NCORES = 8
NCOEF = 5             # Taylor terms n = 0..NCOEF-1
NMOM = NCOEF + 1      # moments W'_0 .. W'_NCOEF
JS = 16               # s-chunk columns per core (128*16 = 2048 s per core)
S_TOTAL = 16384

# tensor_tensor_reduce hard-crashes the exec unit on HW (NRT_EXEC_UNIT_
# UNRECOVERABLE) — keep off.
K_TTR = os.environ.get("K_TTR", "0") == "1"

# params tensor column layout ([128, PCOLS] fp32, single DMA)
PC_E = 0              # cols 0:16  one-hot chunk selector (per-core)
PC_W00 = 16           # conv taps, broadcast down partitions
PC_W01 = 17
PC_W10 = 18
PC_W11 = 19
PC_CBH = 20           # 0.5*conv_b broadcast (tanh bias)
PC_GA = 21            # g scale broadcast
PC_GC = 22            # g bias broadcast
PC_CA = 25            # cols 25:30 : den coeff scales invf_k*2^-k (broadcast)
PC_CB = 30            # cols 30:35 : num coeff scales invf_k*2^-(k+1) (broadcast)
PCOLS = 35


def _emit(ctx: ExitStack, tc: "tile.TileContext", d):
    nc = tc.nc
    pool = ctx.enter_context(tc.tile_pool(name="main", bufs=1))
    psum = ctx.enter_context(tc.tile_pool(name="ps", bufs=1, space="PSUM"))

    def T(name, shape):
        return pool.tile(shape, F32, tag=name, name=name)

    # ---------------- DMAs: one per queue, data first -----------------------
    dataA = T("dataA", [128, 129])
    dataB = T("dataB", [128, 129])
    prm = T("prm", [128, PCOLS])
    nc.sync.dma_start(out=dataA[:, :], in_=d["data"].ap()[0:128, :])
    nc.gpsimd.dma_start(out=dataB[:, :], in_=d["data"].ap()[1:129, :])
    nc.scalar.dma_start(out=prm[:, :], in_=d["params"].ap())

    # ---------------- constants + act-table warmup --------------------------
    onescol = T("onescol", [128, 1])
    ones2d = T("ones2d", [128, 128])
    wacc = T("wacc", [128, NMOM + NCOEF])   # cols 0:6 W'_n, cols 6:11 W'_k+W'_{k+1}
    nc.vector.memset(onescol[:, :], 1.0)
    nc.vector.memset(ones2d[:, :], 1.0)
    nc.vector.memset(wacc[:, 0:1], 128.0)          # W'_0 partial (128*128=S)
    dum = T("dum", [1, 1])
    nc.scalar.activation(dum[:, :], onescol[0:1, 0:1], AF.Tanh, bias=0.0, scale=1.0)

    # ---------------- conv -> pre (vector, serial 4-tap chain) --------------
    c1 = T("c1", [128, 128]); c2 = T("c2", [128, 128])
    c3 = T("c3", [128, 128]); pre = T("pre", [128, 128])
    nc.vector.scalar_tensor_tensor(c1[:, :], dataA[:, 0:128], prm[:, PC_W00:PC_W00 + 1],
                                   dataA[:, 0:128], OP.mult, OP.bypass)
    nc.vector.scalar_tensor_tensor(c2[:, :], dataA[:, 1:129], prm[:, PC_W01:PC_W01 + 1],
                                   c1[:, :], OP.mult, OP.add)
    nc.vector.scalar_tensor_tensor(c3[:, :], dataB[:, 0:128], prm[:, PC_W10:PC_W10 + 1],
                                   c2[:, :], OP.mult, OP.add)
    nc.vector.scalar_tensor_tensor(pre[:, :], dataB[:, 1:129], prm[:, PC_W11:PC_W11 + 1],
                                   c3[:, :], OP.mult, OP.add)

    # ---------------- u = tanh(pre/2 + cb/2) = 2f-1, accum -> W'_1 ----------
    u = T("u", [128, 128])
    nc.scalar.activation(u[:, :], pre[:, :], AF.Tanh,
                         bias=prm[:, PC_CBH:PC_CBH + 1], scale=0.5,
                         accum_out=wacc[:, 1:2])

    # ---------------- moments W'_2..W'_5 (vector) ---------------------------
    u2 = T("u2", [128, 128]); u3 = T("u3", [128, 128])
    s4 = T("s4", [128, 128]); s5 = T("s5", [128, 128])
    if K_TTR:
        nc.vector.tensor_tensor_reduce(u2[:, :], u[:, :], u[:, :], 1.0, 0.0,
                                       OP.mult, OP.add, wacc[:, 2:3])
        nc.vector.tensor_tensor_reduce(u3[:, :], u2[:, :], u[:, :], 1.0, 0.0,
                                       OP.mult, OP.add, wacc[:, 3:4])
        nc.vector.tensor_tensor_reduce(s4[:, :], u2[:, :], u2[:, :], 1.0, 0.0,
                                       OP.mult, OP.add, wacc[:, 4:5])
        nc.vector.tensor_tensor_reduce(s5[:, :], u2[:, :], u3[:, :], 1.0, 0.0,
                                       OP.mult, OP.add, wacc[:, 5:6])
    else:
        nc.vector.tensor_mul(u2[:, :], u[:, :], u[:, :])
        nc.vector.tensor_mul(u3[:, :], u2[:, :], u[:, :])
        nc.vector.tensor_mul(s4[:, :], u2[:, :], u2[:, :])
        nc.vector.tensor_mul(s5[:, :], u2[:, :], u3[:, :])
        nc.vector.reduce_sum(wacc[:, 2:3], u2[:, :], axis=AX.X)
        nc.vector.reduce_sum(wacc[:, 3:4], u3[:, :], axis=AX.X)
        nc.vector.reduce_sum(wacc[:, 4:5], s4[:, :], axis=AX.X)
        nc.vector.reduce_sum(wacc[:, 5:6], s5[:, :], axis=AX.X)
    # pairwise sums W'_k + W'_{k+1} for the Num coefficients
    nc.vector.tensor_add(wacc[:, NMOM:NMOM + NCOEF], wacc[:, 0:NCOEF], wacc[:, 1:NMOM])

    # ---------------- per-core chunk select + g (PE + scalar) ---------------
    chunk_ps = psum.tile([128, JS], F32, tag="chunk", name="chunk")
    nc.tensor.matmul(chunk_ps[:, :], u[:, :], prm[:, PC_E:PC_E + JS],
                     start=True, stop=True)
    g = T("g", [128, JS])
    nc.scalar.activation(g[:, :], chunk_ps[:, :], AF.Identity,
                         bias=prm[:, PC_GC:PC_GC + 1], scale=prm[:, PC_GA:PC_GA + 1])

    # ---------------- broadcast coefficient columns -------------------------
    # ones2d stationary: every output partition gets the column sums of wacc,
    # i.e. bigw[p, n] = W'_n for all p.  cd_k = W'_k*cA_k ; cn_k = (W'_k +
    # W'_{k+1})*cB_k, with cA/cB broadcast columns from the params DMA.
    bigw_ps = psum.tile([128, NMOM + NCOEF], F32, tag="bigw", name="bigw")
    nc.tensor.matmul(bigw_ps[:, :], ones2d[:, :], wacc[:, :], start=True, stop=True)
    coeffb = T("coeffb", [128, 2 * NCOEF])
    nc.vector.tensor_mul(coeffb[:, 0:NCOEF], bigw_ps[:, 0:NCOEF],
                         prm[:, PC_CA:PC_CA + NCOEF])
    nc.vector.tensor_mul(coeffb[:, NCOEF:2 * NCOEF], bigw_ps[:, NMOM:NMOM + NCOEF],
                         prm[:, PC_CB:PC_CB + NCOEF])

    # ---------------- fused Den/Num Horner on [128, 16] ----------------------
    # t-form: t = (t + c)*g each step; the trailing *g cancels in Num/Den.
    td = T("td", [128, JS]); tn = T("tn", [128, JS])
    nc.vector.scalar_tensor_tensor(td[:, :], g[:, :], coeffb[:, NCOEF - 1:NCOEF],
                                   g[:, :], OP.mult, OP.bypass)
    nc.vector.scalar_tensor_tensor(tn[:, :], g[:, :], coeffb[:, 2 * NCOEF - 1:2 * NCOEF],
                                   g[:, :], OP.mult, OP.bypass)
    for k in range(NCOEF - 2, -1, -1):
        nc.vector.scalar_tensor_tensor(td[:, :], td[:, :], coeffb[:, k:k + 1],
                                       g[:, :], OP.add, OP.mult)
        nc.vector.scalar_tensor_tensor(tn[:, :], tn[:, :], coeffb[:, NCOEF + k:NCOEF + k + 1],
                                       g[:, :], OP.add, OP.mult)

    # ---------------- m = Num/Den, partial row sum --------------------------
    rden = T("rden", [128, JS])
    nc.vector.reciprocal(rden[:, :], td[:, :])
    mprod = T("mprod", [128, JS])
    nc.vector.tensor_mul(mprod[:, :], tn[:, :], rden[:, :])
    msum_ps = psum.tile([1, JS], F32, tag="msum", name="msum")
    nc.tensor.matmul(msum_ps[:, :], onescol[:, :], mprod[:, :], start=True, stop=True)
    mrow = T("mrow", [1, JS])
    nc.vector.tensor_copy(mrow[:, :], msum_ps[:, :])
    nc.sync.dma_start(out=d["out"].ap(), in_=mrow[:, :])


def build_nc():
    nc = bacc.Bacc("TRN2", target_bir_lowering=False, debug=False,
                   enable_asserts=False, num_devices=NCORES)
    d = {}
    d["data"] = nc.dram_tensor("data", [129, 129], F32, kind="ExternalInput")
    d["params"] = nc.dram_tensor("params", [128, PCOLS], F32, kind="ExternalInput")
    d["out"] = nc.dram_tensor("out", [1, JS], F32, kind="ExternalOutput")
    with tile.TileContext(nc) as tc:
        with ExitStack() as ctx:
            _emit(ctx, tc, d)
    nc.compile()
    return nc


_NC = None


def _get_nc():
    global _NC
    if _NC is None:
        _NC = build_nc()
    return _NC


def _factorial(n):
    r = 1
    for i in range(2, n + 1):
        r *= i
    return r


def _host_derived(inputs):
    Wq = np.asarray(inputs["Wq"], np.float64)
    Wk = np.asarray(inputs["Wk"], np.float64)
    Wv = np.asarray(inputs["Wv"], np.float64)
    bq = np.asarray(inputs["bq"], np.float64)
    bv = np.asarray(inputs["bv"], np.float64)
    rq = Wq.sum(1); rk = Wk.sum(1); rv = Wv.sum(1)
    A = float(rq @ rk)
    C = float(bq @ rk)
    ga = A / 4.0
    gc = A / 4.0 + C / 2.0
    alpha = rv.sum() / (4.0 * S_TOTAL)
    beta = bv.sum() / 4.0
    return ga, gc, alpha, beta


def make_in_maps(inputs):
    cw = np.asarray(inputs["conv_w"], np.float64)[0, 0]
    cbh = 0.5 * float(np.asarray(inputs["conv_b"], np.float64)[0])
    ga, gc, _, _ = _host_derived(inputs)

    base_p = np.zeros((128, PCOLS), np.float32)
    base_p[:, PC_W00] = cw[0, 0]
    base_p[:, PC_W01] = cw[0, 1]
    base_p[:, PC_W10] = cw[1, 0]
    base_p[:, PC_W11] = cw[1, 1]
    base_p[:, PC_CBH] = cbh
    base_p[:, PC_GA] = ga
    base_p[:, PC_GC] = gc
    for k in range(NCOEF):
        base_p[:, PC_CA + k] = 2.0 ** -k / _factorial(k)
        base_p[:, PC_CB + k] = 2.0 ** -(k + 1) / _factorial(k)

    data = np.ascontiguousarray(inputs["data"], np.float32)
    in_maps = []
    for c in range(NCORES):
        p = base_p.copy()
        p[JS * c + np.arange(JS), np.arange(JS)] = 1.0
        in_maps.append({"data": data, "params": p})
    return in_maps


def run_on_hw(inputs, trace=False, **kw):
    nc = _get_nc()
    res = run_bass_kernel_spmd(nc, make_in_maps(inputs),
                               core_ids=list(range(NCORES)), trace=trace, **kw)
    _, _, alpha, beta = _host_derived(inputs)
    total = np.float64(0.0)
    for r in res.results:
        total += np.asarray(r["out"], np.float64).sum()
    return np.float32(alpha * total + beta), res


def kernel(**inputs) -> np.ndarray:
    out, _ = run_on_hw(inputs, trace=False)
    return out


# revision 13
# speedup vs baseline: 1.5549x; 1.0710x over previous
"""Trainium2 Bass kernel for nn_ConvAttentionHybrid.

Math: the reference broadcasts the conv-sigmoid output f[s] along the embed
dim E, so q/k/v are affine (rank-1) in f.  The softmax weights collapse to
    w[s,t] ~ exp(g[s]*h[t]),   g = A/4*u + (A/4 + C/2),  u = 2h = 2f-1
with u = tanh(z/2) for conv pre-activation z (sigmoid(z)-1/2 = tanh(z/2)/2),
A = rowsum(Wq).rowsum(Wk), C = bq.rowsum(Wk).  With moments W'_n = sum_t u^n:
    Den(s) = sum_n g^n/n! 2^-n W'_n
    Num(s) = sum_n g^n/n! 2^-(n+1) (W'_{n+1} + W'_n)      (= Num_f directly)
    m(s)   = Num/Den,   result = sv_sum*mean_s m(s)/4 + bv_sum/4.
|g*h| <= ~0.5 here so 5 Taylor terms (n=0..4) are exact to ~5e-7, far below
the 2e-2 gate.  Each core computes u and the moments fully (cheap) and
evaluates m(s) for a 2048-row chunk of s selected by a per-core one-hot
matmul; the host applies the final affine and sums the partial outputs.
"""

import os

import numpy as np

from contextlib import ExitStack

import concourse.bass as bass
import concourse.tile as tile
from concourse import bacc, mybir
from concourse.bass_utils import run_bass_kernel_spmd

AF = mybir.ActivationFunctionType
OP = mybir.AluOpType
AX = mybir.AxisListType
F32 = mybir.dt.float32

NCORES = 8
NCOEF = 4             # Taylor terms n = 0..NCOEF-1 (truncation ~2.5e-6 rel)
NMOM = NCOEF + 1      # moments W'_0 .. W'_NCOEF
JS = 16               # s-chunk columns per core (128*16 = 2048 s per core)
S_TOTAL = 16384

# native tensor_tensor_reduce hard-crashes the exec unit on HW
# (NRT_EXEC_UNIT_UNRECOVERABLE); the ant-dve affine_mul_reduce ucode op
# provides the same fused mul+row-sum.
K_AMR = os.environ.get("K_AMR", "1") == "1"

# params tensor column layout ([128, PCOLS] fp32, single DMA)
PC_E = 0              # cols 0:16  one-hot chunk selector (per-core)
PC_W00 = 16           # conv taps, broadcast down partitions
PC_W01 = 17
PC_W10 = 18
PC_W11 = 19
PC_CBH = 20           # 0.5*conv_b broadcast (tanh bias)
PC_GA = 21            # g scale broadcast
PC_GC = 22            # g bias broadcast
PC_CA = 25            # cols 25:30 : den coeff scales invf_k*2^-k (broadcast)
PC_CB = 30            # cols 30:35 : num coeff scales invf_k*2^-(k+1) (broadcast)
PCOLS = 35


def _emit(ctx: ExitStack, tc: "tile.TileContext", d):
    nc = tc.nc
    pool = ctx.enter_context(tc.tile_pool(name="main", bufs=1))
    psum = ctx.enter_context(tc.tile_pool(name="ps", bufs=1, space="PSUM"))

    def T(name, shape):
        return pool.tile(shape, F32, tag=name, name=name)

    # ---------------- DMAs: one per queue, data first -----------------------
    dataA = T("dataA", [128, 129])
    dataB = T("dataB", [128, 129])
    prm = T("prm", [128, PCOLS])
    nc.sync.dma_start(out=dataA[:, :], in_=d["data"].ap()[0:128, :])
    nc.gpsimd.dma_start(out=dataB[:, :], in_=d["data"].ap()[1:129, :])
    nc.scalar.dma_start(out=prm[:, :], in_=d["params"].ap())

    # ---------------- constants + act-table warmup --------------------------
    onescol = T("onescol", [128, 1])
    ones2d = T("ones2d", [128, 128])
    wacc = T("wacc", [128, NMOM + NCOEF])   # cols 0:6 W'_n, cols 6:11 W'_k+W'_{k+1}
    nc.vector.memset(onescol[:, :], 1.0)
    nc.vector.memset(ones2d[:, :], 1.0)
    nc.vector.memset(wacc[:, 0:1], 128.0)          # W'_0 partial (128*128=S)
    dum = T("dum", [1, 1])
    nc.scalar.activation(dum[:, :], onescol[0:1, 0:1], AF.Tanh, bias=0.0, scale=1.0)

    # ---------------- conv -> pre (vector, serial 4-tap chain) --------------
    c1 = T("c1", [128, 128]); c2 = T("c2", [128, 128])
    c3 = T("c3", [128, 128]); pre = T("pre", [128, 128])
    nc.vector.scalar_tensor_tensor(c1[:, :], dataA[:, 0:128], prm[:, PC_W00:PC_W00 + 1],
                                   dataA[:, 0:128], OP.mult, OP.bypass)
    nc.vector.scalar_tensor_tensor(c2[:, :], dataA[:, 1:129], prm[:, PC_W01:PC_W01 + 1],
                                   c1[:, :], OP.mult, OP.add)
    nc.vector.scalar_tensor_tensor(c3[:, :], dataB[:, 0:128], prm[:, PC_W10:PC_W10 + 1],
                                   c2[:, :], OP.mult, OP.add)
    nc.vector.scalar_tensor_tensor(pre[:, :], dataB[:, 1:129], prm[:, PC_W11:PC_W11 + 1],
                                   c3[:, :], OP.mult, OP.add)

    # ---------------- u = tanh(pre/2 + cb/2) = 2f-1, accum -> W'_1 ----------
    u = T("u", [128, 128])
    nc.scalar.activation(u[:, :], pre[:, :], AF.Tanh,
                         bias=prm[:, PC_CBH:PC_CBH + 1], scale=0.5,
                         accum_out=wacc[:, 1:2])

    # ---------------- moments W'_2..W'_4 (vector, fused mul+row-sum) --------
    u2 = T("u2", [128, 128]); u3 = T("u3", [128, 128])
    s4 = T("s4", [128, 128])
    if K_AMR:
        nc.vector.affine_mul_reduce(u2[:, :], wacc[:, 2:3], u[:, :], u[:, :], 1.0, 0.0)
        nc.vector.affine_mul_reduce(u3[:, :], wacc[:, 3:4], u2[:, :], u[:, :], 1.0, 0.0)
        nc.vector.affine_mul_reduce(s4[:, :], wacc[:, 4:5], u2[:, :], u2[:, :], 1.0, 0.0)
    else:
        nc.vector.tensor_mul(u2[:, :], u[:, :], u[:, :])
        nc.vector.tensor_mul(u3[:, :], u2[:, :], u[:, :])
        nc.vector.tensor_mul(s4[:, :], u2[:, :], u2[:, :])
        nc.vector.reduce_sum(wacc[:, 2:3], u2[:, :], axis=AX.X)
        nc.vector.reduce_sum(wacc[:, 3:4], u3[:, :], axis=AX.X)
        nc.vector.reduce_sum(wacc[:, 4:5], s4[:, :], axis=AX.X)
    # pairwise sums W'_k + W'_{k+1} for the Num coefficients
    nc.vector.tensor_add(wacc[:, NMOM:NMOM + NCOEF], wacc[:, 0:NCOEF], wacc[:, 1:NMOM])

    # ---------------- per-core chunk select + g (PE + scalar) ---------------
    chunk_ps = psum.tile([128, JS], F32, tag="chunk", name="chunk")
    nc.tensor.matmul(chunk_ps[:, :], u[:, :], prm[:, PC_E:PC_E + JS],
                     start=True, stop=True)
    g = T("g", [128, JS])
    nc.scalar.activation(g[:, :], chunk_ps[:, :], AF.Identity,
                         bias=prm[:, PC_GC:PC_GC + 1], scale=prm[:, PC_GA:PC_GA + 1])

    # ---------------- broadcast coefficient columns -------------------------
    # ones2d stationary: every output partition gets the column sums of wacc,
    # i.e. bigw[p, n] = W'_n for all p.  cd_k = W'_k*cA_k ; cn_k = (W'_k +
    # W'_{k+1})*cB_k, with cA/cB broadcast columns from the params DMA.
    bigw_ps = psum.tile([128, NMOM + NCOEF], F32, tag="bigw", name="bigw")
    nc.tensor.matmul(bigw_ps[:, :], ones2d[:, :], wacc[:, :], start=True, stop=True)
    coeffb = T("coeffb", [128, 2 * NCOEF])
    nc.vector.tensor_mul(coeffb[:, 0:NCOEF], bigw_ps[:, 0:NCOEF],
                         prm[:, PC_CA:PC_CA + NCOEF])
    nc.vector.tensor_mul(coeffb[:, NCOEF:2 * NCOEF], bigw_ps[:, NMOM:NMOM + NCOEF],
                         prm[:, PC_CB:PC_CB + NCOEF])

    # ---------------- fused Den/Num Horner on [128, 16] ----------------------
    # t-form: t = (t + c)*g each step; the trailing *g cancels in Num/Den.
    td = T("td", [128, JS]); tn = T("tn", [128, JS])
    nc.vector.scalar_tensor_tensor(td[:, :], g[:, :], coeffb[:, NCOEF - 1:NCOEF],
                                   g[:, :], OP.mult, OP.bypass)
    nc.vector.scalar_tensor_tensor(tn[:, :], g[:, :], coeffb[:, 2 * NCOEF - 1:2 * NCOEF],
                                   g[:, :], OP.mult, OP.bypass)
    for k in range(NCOEF - 2, -1, -1):
        nc.vector.scalar_tensor_tensor(td[:, :], td[:, :], coeffb[:, k:k + 1],
                                       g[:, :], OP.add, OP.mult)
        nc.vector.scalar_tensor_tensor(tn[:, :], tn[:, :], coeffb[:, NCOEF + k:NCOEF + k + 1],
                                       g[:, :], OP.add, OP.mult)

    # ---------------- m = Num/Den, partial row sum --------------------------
    rden = T("rden", [128, JS])
    nc.vector.reciprocal(rden[:, :], td[:, :])
    mprod = T("mprod", [128, JS])
    nc.vector.tensor_mul(mprod[:, :], tn[:, :], rden[:, :])
    msum_ps = psum.tile([1, JS], F32, tag="msum", name="msum")
    nc.tensor.matmul(msum_ps[:, :], onescol[:, :], mprod[:, :], start=True, stop=True)
    mrow = T("mrow", [1, JS])
    nc.vector.tensor_copy(mrow[:, :], msum_ps[:, :])
    nc.sync.dma_start(out=d["out"].ap(), in_=mrow[:, :])


def build_nc():
    nc = bacc.Bacc("TRN2", target_bir_lowering=False, debug=False,
                   enable_asserts=False, num_devices=NCORES)
    d = {}
    d["data"] = nc.dram_tensor("data", [129, 129], F32, kind="ExternalInput")
    d["params"] = nc.dram_tensor("params", [128, PCOLS], F32, kind="ExternalInput")
    d["out"] = nc.dram_tensor("out", [1, JS], F32, kind="ExternalOutput")
    with tile.TileContext(nc) as tc:
        with ExitStack() as ctx:
            _emit(ctx, tc, d)
    nc.compile()
    return nc


_NC = None


def _get_nc():
    global _NC
    if _NC is None:
        _NC = build_nc()
    return _NC


def _factorial(n):
    r = 1
    for i in range(2, n + 1):
        r *= i
    return r


def _host_derived(inputs):
    Wq = np.asarray(inputs["Wq"], np.float64)
    Wk = np.asarray(inputs["Wk"], np.float64)
    Wv = np.asarray(inputs["Wv"], np.float64)
    bq = np.asarray(inputs["bq"], np.float64)
    bv = np.asarray(inputs["bv"], np.float64)
    rq = Wq.sum(1); rk = Wk.sum(1); rv = Wv.sum(1)
    A = float(rq @ rk)
    C = float(bq @ rk)
    ga = A / 4.0
    gc = A / 4.0 + C / 2.0
    alpha = rv.sum() / (4.0 * S_TOTAL)
    beta = bv.sum() / 4.0
    return ga, gc, alpha, beta


def make_in_maps(inputs):
    cw = np.asarray(inputs["conv_w"], np.float64)[0, 0]
    cbh = 0.5 * float(np.asarray(inputs["conv_b"], np.float64)[0])
    ga, gc, _, _ = _host_derived(inputs)

    base_p = np.zeros((128, PCOLS), np.float32)
    base_p[:, PC_W00] = cw[0, 0]
    base_p[:, PC_W01] = cw[0, 1]
    base_p[:, PC_W10] = cw[1, 0]
    base_p[:, PC_W11] = cw[1, 1]
    base_p[:, PC_CBH] = cbh
    base_p[:, PC_GA] = ga
    base_p[:, PC_GC] = gc
    for k in range(NCOEF):
        base_p[:, PC_CA + k] = 2.0 ** -k / _factorial(k)
        base_p[:, PC_CB + k] = 2.0 ** -(k + 1) / _factorial(k)

    data = np.ascontiguousarray(inputs["data"], np.float32)
    in_maps = []
    for c in range(NCORES):
        p = base_p.copy()
        p[JS * c + np.arange(JS), np.arange(JS)] = 1.0
        in_maps.append({"data": data, "params": p})
    return in_maps


def run_on_hw(inputs, trace=False, **kw):
    nc = _get_nc()
    res = run_bass_kernel_spmd(nc, make_in_maps(inputs),
                               core_ids=list(range(NCORES)), trace=trace, **kw)
    _, _, alpha, beta = _host_derived(inputs)
    total = np.float64(0.0)
    for r in res.results:
        total += np.asarray(r["out"], np.float64).sum()
    return np.float32(alpha * total + beta), res


def kernel(**inputs) -> np.ndarray:
    out, _ = run_on_hw(inputs, trace=False)
    return out
